# revision 1
# baseline (speedup 1.0000x reference)
"""LSTM encoder (B=64, S=512, E=H=1024) on 8 trn2 NeuronCores.

Strategy:
  - Tensor-parallel over the 4H gate dimension: each core owns 128 hidden
    channels (x4 gates = 512 gate rows), the full batch (64), and the full
    sequence.
  - Phase 1 (parallel): embedding gather via dma_gather(transpose=True)
    directly into X^T layout, then gx = W_ih_local @ X^T for all 32768
    tokens, stored to DRAM as bf16.
  - Phase 2 (recurrence): 512 sequential steps. Per step: gates.T =
    identity-matmul(gx_t) + sum_k W_hh_chunk @ h_chunk (PSUM accumulate),
    sigmoid/tanh on ScalarE, cell update on VectorE, then the new local
    h slice (128 ch x 64 batch, bf16) is pushed to all 7 peers' SBUF via
    remote_dma_broadcast (XOR-relative addressing, so the same SPMD
    program works on every core; per-core W_hh is XOR-permuted on host).
  - h lives in [-1,1]: bf16 exchange; c stays fp32 on-core.

Self-contained: hardcodes all shapes; host-side prep is numpy only.
"""

import os
import sys

sys.path.insert(0, "/opt/trn_rl_repo")

import numpy as np
import ml_dtypes

import concourse.bass as bass
import concourse.bacc as bacc
import concourse.mybir as mybir

BF16 = ml_dtypes.bfloat16
AF = mybir.ActivationFunctionType
dt = mybir.dt

# problem constants
VOCAB, EMB, HID = 32000, 1024, 1024
B = 64
S = 512
CORES = 8
KC = 8            # contraction chunks of 128
NCHUNK = 4        # gate chunks per core (order: g, i, f, o)
G = NCHUNK * 128  # 512 gate rows per core
NT = 512          # tokens per phase-1 tile
TPT = NT // B     # timesteps per phase-1 tile (8)
# pytorch gate blocks in W rows: i, f, g, o ; our chunk order: g, i, f, o
CHUNK_TO_BLOCK = [2, 0, 1, 3]

# Logical(replica) -> physical TPB mapping on trn2 (driver V0 table, the
# per-chip base offset cancels): upper-die pairs are swapped. The remote
# broadcast's relative (0, d) dest goes to physical (p ^ d), so replica r's
# slot d receives from replica PERM[r][d] = m(m(r) ^ d). HW-verified by a
# probe kernel (each core broadcast its id; see T table in dev notes).
_M = [0, 1, 2, 3, 6, 7, 4, 5]


def sender_at_slot(r, d):
    return _M[_M[r] ^ d]


def build(nc_steps=S, exchange="remote", nbcast=7, wait_rsem=True):
    """Emit the SPMD bass program (identical on all 8 cores)."""
    nsteps = nc_steps
    TT = B * nsteps // NT  # number of phase-1 token tiles
    assert B * nsteps % NT == 0

    nc = bacc.Bacc(None, target_bir_lowering=False)

    # ---- kernel I/O (per core) ----
    emb_d = nc.declare_dram_parameter("emb16", [VOCAB, EMB], dt.bfloat16, isOutput=False)
    idx_d = nc.declare_dram_parameter("idx", [TT, 128, NT // 16], dt.int16, isOutput=False)
    wih_d = nc.declare_dram_parameter("w_ih", [128, KC * G], dt.bfloat16, isOutput=False)
    whh_d = nc.declare_dram_parameter("w_hh", [128, KC * G], dt.bfloat16, isOutput=False)
    ident_d = nc.declare_dram_parameter("ident", [128, 128], dt.bfloat16, isOutput=False)
    gbias_d = nc.declare_dram_parameter("gbias", [128, NCHUNK], dt.float32, isOutput=False)
    out_d = nc.declare_dram_parameter("out", [2, 128, B], dt.float32, isOutput=True)

    # ---- DRAM scratch ----
    gx_d = nc.dram_tensor("gx", [128, nsteps, NCHUNK * B], dt.bfloat16)
    bar_in = nc.dram_tensor("bar_in", [128, 4], dt.float32)
    bar_out = nc.dram_tensor("bar_out", [128, 4], dt.float32, addr_space="Shared")

    # ---- semaphores ----
    cc_sem = nc.alloc_semaphore("cc_sem")
    bar_sem = nc.alloc_semaphore("bar_sem")
    bardma_sem = nc.alloc_semaphore("bardma_sem")
    wload = nc.alloc_semaphore("wload")
    g_sem = [nc.alloc_semaphore("g_sem0"), nc.alloc_semaphore("g_sem1")]
    mm1 = nc.alloc_semaphore("mm1")
    cp_sem = nc.alloc_semaphore("cp_sem")
    st_sem = [nc.alloc_semaphore("st_sem0"), nc.alloc_semaphore("st_sem1")]
    gxd = [nc.alloc_semaphore("gxd0"), nc.alloc_semaphore("gxd1")]
    idm = nc.alloc_semaphore("idm")
    mmr = nc.alloc_semaphore("mmr")
    act_s = nc.alloc_semaphore("act_s")
    dve_s = nc.alloc_semaphore("dve_s")
    prep_s = nc.alloc_semaphore("prep_s")
    # parity-split: exchange e increments index (e+1)%2; the 2-step pipeline
    # separation guarantees no cross-exchange mixing within one parity chain.
    rsem = [nc.alloc_semaphore("rsem0"), nc.alloc_semaphore("rsem1")]  # +2 x7 per exchange
    lsem = [nc.alloc_semaphore("lsem0"), nc.alloc_semaphore("lsem1")]  # +16 x7 per exchange
    fin = nc.alloc_semaphore("fin")

    from contextlib import ExitStack

    with ExitStack() as ctx:
        sb = lambda name, shape, d: ctx.enter_context(nc.sbuf_tensor(name, shape, d))
        idx_sb = sb("idx_sb", [128, TT * (NT // 16)], dt.int16)
        wih_sb = sb("wih_sb", [128, KC * G], dt.bfloat16)
        whh_sb = sb("whh_sb", [128, KC * G], dt.bfloat16)
        ident_sb = sb("ident_sb", [128, 128], dt.bfloat16)
        gbias_sb = sb("gbias_sb", [128, NCHUNK], dt.float32)
        xt = [sb(f"xt{i}", [128, KC, NT], dt.bfloat16) for i in range(2)]
        stage = [sb(f"stage{i}", [128, TPT * NCHUNK * B], dt.bfloat16) for i in range(2)]
        hg = [sb(f"hg{i}", [128, CORES * B], dt.bfloat16) for i in range(2)]
        gxt = [sb(f"gxt{i}", [128, NCHUNK * B], dt.bfloat16) for i in range(2)]
        sg = sb("sg", [128, NCHUNK * B], dt.float32)
        ig_sb = sb("ig_sb", [128, B], dt.float32)
        fc_sb = sb("fc_sb", [128, B], dt.float32)
        thc_sb = sb("thc_sb", [128, B], dt.float32)
        c_sb = sb("c_sb", [128, B], dt.float32)
        hout_sb = sb("hout_sb", [128, B], dt.float32)
        bar_sb = sb("bar_sb", [128, 4], dt.float32)
        # PSUM: 8 tensors of [128, 512] fp32 = 8 full banks
        psum = [
            ctx.enter_context(nc.psum_tensor(f"ps{i}", [128, 512], dt.float32))
            for i in range(8)
        ]
        block = ctx.enter_context(nc.Block())

        NIDX = NT // 16  # idx columns per tile

        # =========== SYNC engine: weight loads, phase-1 stores, ===========
        # =========== phase-2 gx prefetch, final output            ===========
        @block.sync
        def _(sy):
            # preload constants (HWDGE, FIFO order)
            sy.dma_start(
                out=idx_sb.ap().rearrange("p (t c) -> p t c", t=TT),
                in_=idx_d.ap().rearrange("t p c -> p t c"),
            ).then_inc(wload, 16)
            sy.dma_start(out=wih_sb[:, :], in_=wih_d[:, :]).then_inc(wload, 16)
            sy.dma_start(out=whh_sb[:, :], in_=whh_d[:, :]).then_inc(wload, 16)
            sy.dma_start(out=ident_sb[:, :], in_=ident_d[:, :]).then_inc(wload, 16)
            sy.dma_start(out=gbias_sb[:, :], in_=gbias_d[:, :]).then_inc(wload, 16)

            # phase-1 stores
            for tau in range(TT):
                sy.wait_ge(cp_sem, 4 * tau + 4)
                sy.dma_start(
                    out=gx_d[:, TPT * tau : TPT * (tau + 1), :],
                    in_=stage[tau % 2].ap().rearrange("p (t e) -> p t e", t=TPT),
                ).then_inc(st_sem[tau % 2], 16)

            # phase-2 gx prefetch: first two, then rolling
            sy.dma_start(out=gxt[0][:, :], in_=gx_d[:, 0, :]).then_inc(gxd[0], 16)
            if nsteps > 1:
                sy.dma_start(out=gxt[1][:, :], in_=gx_d[:, 1, :]).then_inc(gxd[1], 16)
            for t in range(2, nsteps):
                sy.wait_ge(idm, t - 1)
                sy.dma_start(out=gxt[t % 2][:, :], in_=gx_d[:, t, :]).then_inc(gxd[t % 2], 16)

            # final outputs
            sy.wait_ge(dve_s, 1 + 4 * nsteps)
            sy.dma_start(out=out_d[0, :, :], in_=hout_sb[:, :]).then_inc(fin, 16)
            sy.dma_start(out=out_d[1, :, :], in_=c_sb[:, :]).then_inc(fin, 16)
            sy.wait_ge(fin, 32)

        # =========== GPSIMD: barrier, gathers, h broadcast ===========
        @block.gpsimd
        def _(gp):
            # cross-core barrier: protects remote-sem increments from
            # racing a peer's kernel-start semaphore init.
            gp.memset(bar_sb[:, :], 0.0).then_inc(bar_sem, 1)
            gp.wait_ge(bar_sem, 1)
            gp.dma_start(out=bar_in[:, :], in_=bar_sb[:, :]).then_inc(bardma_sem, 16)
            gp.wait_ge(bardma_sem, 16)
            gp.collective_compute(
                "AllReduce",
                mybir.AluOpType.add,
                ins=[bar_in.ap().opt()],
                outs=[bar_out.ap().opt()],
                replica_groups=[list(range(CORES))],
            ).then_inc(cc_sem, 1)

            # phase-1 embedding gathers (transposing: out[p, k, j] = emb[idx_j, 128k+p])
            gp.wait_ge(wload, 80)  # constants loaded (incl. idx_sb)
            for tau in range(TT):
                if tau >= 2:
                    gp.wait_ge(mm1, 4 * (tau - 2) + 4)  # xt buffer free
                gp.dma_gather(
                    out_ap=xt[tau % 2][:, :, :],
                    in_ap=emb_d[:, :],
                    idxs_ap=idx_sb[:, NIDX * tau : NIDX * (tau + 1)],
                    num_idxs=NT,
                    num_idxs_reg=NT,
                    elem_size=EMB,
                    transpose=True,
                ).then_inc(g_sem[tau % 2], 16)

            # phase-2 h exchange: 7 broadcast preps + 1 trigger per step
            if exchange == "remote":
                gp.wait_ge(cc_sem, 1)
                for t in range(nsteps - 1):
                    po = (t + 1) % 2  # parity of the buffer holding h(t)
                    for d in range(1, 1 + nbcast):
                        rd = [None] * CORES
                        rd[d] = (0, d)
                        gp.remote_dma_broadcast(
                            out_ap=hg[po][:, B * d : B * (d + 1)],
                            in_ap=hg[po][:, 0:B],
                            remote_sem=rsem[po],
                            local_sem=lsem[po],
                            rdests=rd,
                        ).then_inc(prep_s, 1)
                    gp.wait_ge(prep_s, nbcast * (t + 1))
                    gp.wait_ge(dve_s, 1 + 4 * t + 4)  # h(t) written
                    gp.trigger_dma(count=nbcast)

        # =========== TENSOR engine ===========
        @block.tensor
        def _(te):
            te.wait_ge(wload, 80)
            # ---- phase 1 ----
            for tau in range(TT):
                te.wait_ge(g_sem[tau % 2], 16 * (tau // 2 + 1))
                for cb in range(NCHUNK):
                    pb = psum[(tau % 2) * 4 + cb]
                    if tau >= 2:
                        te.wait_ge(cp_sem, 4 * (tau - 2) + cb + 1)
                    for k in range(KC):
                        mm = te.matmul(
                            pb[:, :],
                            lhsT=wih_sb[:, G * k + 128 * cb : G * k + 128 * (cb + 1)],
                            rhs=xt[tau % 2][:, k, :],
                            start=(k == 0),
                            stop=(k == KC - 1),
                        )
                    mm.then_inc(mm1, 1)

            # ---- phase 2 ----
            for t in range(nsteps):
                P = t % 2
                # identity-matmul loads gx_t into psum (one per gate bank)
                # (first two prefetches are unordered w.r.t. each other)
                te.wait_ge(gxd[t % 2], 16 * (t // 2 + 1))
                if t < 2:
                    te.wait_ge(cp_sem, 4 * TT)  # phase-1 copies fully drained
                else:
                    te.wait_ge(act_s, 5 * (t - 2) + 4)  # psum parity reuse
                for cb in range(NCHUNK):
                    mm = te.matmul(
                        psum[P * 4 + cb][:, 0:B],
                        lhsT=ident_sb[:, :],
                        rhs=gxt[P][:, B * cb : B * (cb + 1)],
                        start=True,
                        stop=(t == 0),
                    )
                    if cb == NCHUNK - 1:
                        mm.then_inc(idm, 1)
                if t >= 1:
                    te.wait_ge(dve_s, 1 + 4 * t)  # own h slice in hg[P][:, 0:B]
                    if exchange == "remote" and wait_rsem:
                        te.wait_ge(rsem[t % 2], 2 * nbcast * ((t + 1) // 2))
                    for cb in range(NCHUNK):
                        for d in range(CORES):
                            mm = te.matmul(
                                psum[P * 4 + cb][:, 0:B],
                                lhsT=whh_sb[:, G * d + 128 * cb : G * d + 128 * (cb + 1)],
                                rhs=hg[P][:, B * d : B * (d + 1)],
                                start=False,
                                stop=(d == CORES - 1),
                            )
                        mm.then_inc(mmr, 1)

        # =========== SCALAR engine (ACT) ===========
        @block.scalar
        def _(sc):
            sc.wait_ge(wload, 80)
            # ---- phase 1: psum -> stage (bf16 cast) ----
            for tau in range(TT):
                for cb in range(NCHUNK):
                    sc.wait_ge(mm1, 4 * tau + cb + 1)
                    if tau >= 2:
                        sc.wait_ge(st_sem[tau % 2], 16 * (tau // 2))  # stage free
                    src = psum[(tau % 2) * 4 + cb].ap().rearrange("p (t b) -> p t b", t=TPT)
                    dst = stage[tau % 2].ap().rearrange(
                        "p (t e b) -> p t e b", t=TPT, e=NCHUNK
                    )[:, :, cb, :]
                    sc.activation(dst, src, AF.Copy).then_inc(cp_sem, 1)

            # ---- phase 2 activations ----
            # chunk order: 0=g(tanh), 1=i, 2=f, 3=o (sigmoid); then tanh(c)
            for t in range(nsteps):
                P = t % 2
                for cb in range(NCHUNK):
                    if t == 0:
                        sc.wait_ge(idm, 1)
                    else:
                        sc.wait_ge(mmr, 4 * (t - 1) + cb + 1)
                    fn = AF.Tanh if cb == 0 else AF.Sigmoid
                    sc.activation(
                        sg[:, B * cb : B * (cb + 1)],
                        psum[P * 4 + cb][:, 0:B],
                        fn,
                        bias=gbias_sb[:, cb : cb + 1],
                    ).then_inc(act_s, 1)
                sc.wait_ge(dve_s, 1 + 4 * t + 3)  # c updated
                sc.activation(thc_sb[:, :], c_sb[:, :], AF.Tanh).then_inc(act_s, 1)

        # =========== VECTOR engine (DVE) ===========
        @block.vector
        def _(ve):
            ve.memset(c_sb[:, :], 0.0).then_inc(dve_s, 1)
            for t in range(nsteps):
                Pn = (t + 1) % 2
                ve.wait_ge(act_s, 5 * t + 2)
                ve.tensor_mul(ig_sb[:, :], sg[:, B : 2 * B], sg[:, 0:B]).then_inc(dve_s, 1)
                ve.wait_ge(act_s, 5 * t + 3)
                # c_sb RAW from previous step's update (or the memset)
                ve.wait_ge(dve_s, max(1, 1 + 4 * (t - 1) + 3))
                ve.tensor_mul(fc_sb[:, :], sg[:, 2 * B : 3 * B], c_sb[:, :]).then_inc(dve_s, 1)
                ve.wait_ge(dve_s, 1 + 4 * t + 2)  # ig, fc writebacks landed
                ve.tensor_add(c_sb[:, :], ig_sb[:, :], fc_sb[:, :]).then_inc(dve_s, 1)
                ve.wait_ge(act_s, 5 * t + 5)
                if t == nsteps - 1:
                    ve.tensor_mul(hout_sb[:, :], sg[:, 3 * B : 4 * B], thc_sb[:, :]).then_inc(dve_s, 1)
                else:
                    if t >= 2 and exchange == "remote":
                        ve.wait_ge(lsem[(t + 1) % 2], 16 * nbcast * (t // 2))
                    ve.tensor_mul(hg[Pn][:, 0:B], sg[:, 3 * B : 4 * B], thc_sb[:, :]).then_inc(dve_s, 1)

    nc.compile()
    return nc


# ---------------------------------------------------------------------------
# host-side input prep
# ---------------------------------------------------------------------------

def prepare_in_maps(source, emb, W_ih, W_hh, b_ih, b_hh, nsteps=S):
    source = np.asarray(source)
    emb = np.asarray(emb, np.float32)
    W_ih = np.asarray(W_ih, np.float32)
    W_hh = np.asarray(W_hh, np.float32)
    b = np.asarray(b_ih, np.float32) + np.asarray(b_hh, np.float32)

    TT = B * nsteps // NT
    emb16 = emb.astype(BF16)
    ident = np.eye(128, dtype=BF16)

    # indices, wrapped: idx[tau, p, s] = source[b, TPT*tau + t'] with
    # j = s*16 + (p % 16), t' = j // 64, b = j % 64
    idx = np.zeros([TT, 128, NT // 16], np.int16)
    j = np.arange(NT)
    tprime, bb = j // B, j % B
    for tau in range(TT):
        ids = source[bb, TPT * tau + tprime].astype(np.int16)  # [NT]
        wrapped = ids.reshape(NT // 16, 16).T  # [16, NT//16]
        idx[tau] = np.tile(wrapped, (8, 1))

    in_maps = []
    H = HID
    for j_core in range(CORES):
        rows = np.concatenate(
            [
                np.arange(CHUNK_TO_BLOCK[cb] * H + 128 * j_core,
                          CHUNK_TO_BLOCK[cb] * H + 128 * (j_core + 1))
                for cb in range(NCHUNK)
            ]
        )
        Wi = W_ih[rows]  # [512, 1024]
        Wh = W_hh[rows]
        bi = b[rows]  # [512]

        # w_ih[p, G*k + 128*cb + m] = Wi[128*cb + m, 128*k + p]
        wi4 = Wi.reshape(NCHUNK, 128, KC, 128)          # [cb, m, k, p]
        wih = np.transpose(wi4, (3, 2, 0, 1)).reshape(128, KC * G).astype(BF16)

        # w_hh with XOR-permuted k chunks: position d holds chunk (j_core ^ d)
        wh4 = Wh.reshape(NCHUNK, 128, KC, 128)          # [cb, m, k, p]
        wh4p = wh4[:, :, [sender_at_slot(j_core, d) for d in range(KC)], :]
        whh = np.transpose(wh4p, (3, 2, 0, 1)).reshape(128, KC * G).astype(BF16)

        gbias = bi.reshape(NCHUNK, 128).T.copy().astype(np.float32)  # [128, 4]

        in_maps.append(
            {
                "emb16": emb16,
                "idx": idx,
                "w_ih": wih,
                "w_hh": whh,
                "ident": ident,
                "gbias": gbias,
            }
        )
    return in_maps


_BUILD_CACHE = {}


def _get_nc(nsteps=S, exchange="remote"):
    key = (nsteps, exchange)
    if key not in _BUILD_CACHE:
        _BUILD_CACHE[key] = build(nsteps, exchange)
    return _BUILD_CACHE[key]


def kernel(source, emb, W_ih, W_hh, b_ih, b_hh, _trace=False):
    from concourse.bass_utils import run_bass_kernel_spmd

    nc = _get_nc()
    in_maps = prepare_in_maps(source, emb, W_ih, W_hh, b_ih, b_hh)
    res = run_bass_kernel_spmd(nc, in_maps, core_ids=list(range(CORES)), trace=_trace)
    outs = [res.results[i]["out"] for i in range(CORES)]  # each [2, 128, B]
    h = np.concatenate([o[0].T for o in outs], axis=1)  # [B, 8*128]
    c = np.concatenate([o[1].T for o in outs], axis=1)
    out = np.stack([h, c]).astype(np.float32)
    if _trace:
        return out, res
    return out


# ---------------------------------------------------------------------------
# dev: multi-core simulation on a reduced problem
# ---------------------------------------------------------------------------

def _simulate(nsteps=8, exchange="remote", check_with_hw=False):
    from concourse import bass_interp, libnrt

    # no /dev/neuron on the axon client: fake the driver's logical->physical
    # NC map with the standard trn2 XOR-4 die-flip table (any XOR-affine
    # bijection preserves the kernel's XOR-relative addressing scheme).
    libnrt.get_trn2_nc_mapping.cache_clear()
    libnrt.nc_to_real_nc.cache_clear()
    libnrt.get_trn2_nc_mapping.__wrapped__.__globals__  # noqa: ensure attr exists
    fake_map = {(d, i): _M[i] for d in range(16) for i in range(8)}
    libnrt.get_trn2_nc_mapping = lambda: fake_map
    libnrt.nc_to_real_nc = lambda dev, i: fake_map[(dev, i)]
    bass_interp.nc_to_real_nc = libnrt.nc_to_real_nc
    bass_interp.pnc_id_to_device_and_real_nc_index = (
        lambda core_id: (core_id // 8, fake_map[(core_id // 8, core_id % 8)])
    )
    fake_rid = {d: d for d in range(16)}
    libnrt.get_device_id_to_routing_id_mapping = lambda: fake_rid
    bass_interp.get_device_id_to_routing_id_mapping = lambda: fake_rid

    rng = np.random.default_rng(0)
    source = rng.integers(0, VOCAB, (B, nsteps)).astype(np.int32)
    emb = rng.standard_normal((VOCAB, EMB), np.float32)
    W_ih = (rng.standard_normal((4 * HID, EMB), np.float32) / np.sqrt(EMB)).astype(np.float32)
    W_hh = (rng.standard_normal((4 * HID, HID), np.float32) / np.sqrt(HID)).astype(np.float32)
    b_ih = np.zeros(4 * HID, np.float32)
    b_hh = np.zeros(4 * HID, np.float32)

    nc = build(nsteps, exchange)
    in_maps = prepare_in_maps(source, emb, W_ih, W_hh, b_ih, b_hh, nsteps)

    sim = bass_interp.MultiCoreSim(nc, CORES)
    for i in range(CORES):
        for k, v in in_maps[i].items():
            sim.cores[i].tensor(k)[:] = v
    sim.simulate(check_with_hw=check_with_hw)

    outs = [
        np.array(sim.cores[i].mem_tensor("out")).reshape(2, 128, B)
        for i in range(CORES)
    ]
    h = np.concatenate([o[0].T for o in outs], axis=1)
    c = np.concatenate([o[1].T for o in outs], axis=1)
    actual = np.stack([h, c])

    # numpy reference
    X = emb[source]  # [B, S, E]
    hh = np.zeros((B, HID), np.float32)
    cc = np.zeros((B, HID), np.float32)
    for t in range(nsteps):
        gates = X[:, t, :] @ W_ih.T + hh @ W_hh.T + b_ih + b_hh
        i_, f_, g_, o_ = np.split(gates, 4, axis=-1)
        i_ = 1 / (1 + np.exp(-i_))
        f_ = 1 / (1 + np.exp(-f_))
        g_ = np.tanh(g_)
        o_ = 1 / (1 + np.exp(-o_))
        cc = f_ * cc + i_ * g_
        hh = o_ * np.tanh(cc)
    expected = np.stack([hh, cc])
    err = np.abs(actual - expected).max() / np.abs(expected).max()
    times = [sim.cores[i].time for i in range(CORES)]
    print(f"sim nsteps={nsteps} absmax_rel_err={err:.3e} sim_time_ns={max(times)}")
    return err


if __name__ == "__main__":
    ns = int(sys.argv[1]) if len(sys.argv) > 1 else 8
    ex = sys.argv[2] if len(sys.argv) > 2 else "remote"
    _simulate(ns, ex)



# revision 2
# speedup vs baseline: 2.4324x; 2.4324x over previous
"""LSTM encoder (B=64, S=512, E=H=1024) on 8 trn2 NeuronCores — v2.

v2 exchange: ONE remote_dma_broadcast per step (vs 7 in v1). The
broadcast targets all 7 peers (relative dests (0,1)..(0,7)); the landing
offset inside each receiver's gather buffer comes from a POOL register
loaded per-core (64*logical_id), so every sender lands in its own slot —
sender-indexed addressing without per-core programs. A core's own slice
never crosses the fabric: it is consumed from the send buffer via a 9th
W_hh contraction chunk, and the (never-written, zero-initialized) own
slot is neutralized with zero weights.

Layout per core (gate-dim tensor parallel, as v1):
  - 512 gate rows (4 gates x 128 hidden channels), full batch 64.
  - Phase 1: dma_gather embedding -> X^T tiles; gx = W_ih_local @ X^T
    streamed to DRAM as bf16.
  - Phase 2: per step: identity-matmul loads gx_t into PSUM; 9x4 W_hh
    matmuls accumulate (8 hg slots + hsend); sigmoid/tanh on ACT; cell
    update on DVE; h slice -> hsend[t%2]; one broadcast -> peers'
    hg[t%4] at slot 64*myid.
"""

import os
import sys

sys.path.insert(0, "/opt/trn_rl_repo")

import numpy as np
import ml_dtypes

import concourse.bass as bass
import concourse.bacc as bacc
import concourse.mybir as mybir
from concourse.ap import AP

BF16 = ml_dtypes.bfloat16
AF = mybir.ActivationFunctionType
dt = mybir.dt

VOCAB, EMB, HID = 32000, 1024, 1024
B = 64
S = 512
CORES = 8
KC = 8            # contraction chunks of 128 (phase 1)
KH = 9            # phase-2 W_hh chunks: 8 hg slots + own (hsend)
NCHUNK = 4        # gate chunks per core (order: g, i, f, o)
G = NCHUNK * 128  # 512 gate rows per core
NT = 512          # tokens per phase-1 tile
TPT = NT // B     # timesteps per phase-1 tile (8)
# pytorch gate blocks in W rows: i, f, g, o ; our chunk order: g, i, f, o
CHUNK_TO_BLOCK = [2, 0, 1, 3]


def build(nc_steps=S, wait_rsem=True, wait_h=True, wait_gx=True):
    nsteps = nc_steps
    TT = B * nsteps // NT
    assert B * nsteps % NT == 0

    nc = bacc.Bacc(None, target_bir_lowering=False)

    # ---- kernel I/O (per core) ----
    emb_d = nc.declare_dram_parameter("emb16", [VOCAB, EMB], dt.bfloat16, isOutput=False)
    idx_d = nc.declare_dram_parameter("idx", [TT, 128, NT // 16], dt.int16, isOutput=False)
    wih_d = nc.declare_dram_parameter("w_ih", [128, KC * G], dt.bfloat16, isOutput=False)
    whh_d = nc.declare_dram_parameter("w_hh", [128, KH * G], dt.bfloat16, isOutput=False)
    ident_d = nc.declare_dram_parameter("ident", [128, 128], dt.bfloat16, isOutput=False)
    gbias_d = nc.declare_dram_parameter("gbias", [128, NCHUNK], dt.float32, isOutput=False)
    slot_d = nc.declare_dram_parameter("slot", [1, 4], dt.int32, isOutput=False)
    out_d = nc.declare_dram_parameter("out", [2, 128, B], dt.float32, isOutput=True)

    # ---- DRAM scratch ----
    gx_d = nc.dram_tensor("gx", [128, nsteps, NCHUNK * B], dt.bfloat16)
    bar_in = nc.dram_tensor("bar_in", [128, 4], dt.float32)
    bar_out = nc.dram_tensor("bar_out", [128, 4], dt.float32, addr_space="Shared")

    # ---- semaphores ----
    cc_sem = nc.alloc_semaphore("cc_sem")
    bar_sem = nc.alloc_semaphore("bar_sem")
    bardma_sem = nc.alloc_semaphore("bardma_sem")
    wload = nc.alloc_semaphore("wload")
    g_sem = [nc.alloc_semaphore("g_sem0"), nc.alloc_semaphore("g_sem1")]
    mm1 = nc.alloc_semaphore("mm1")
    cp_sem = nc.alloc_semaphore("cp_sem")
    st_sem = [nc.alloc_semaphore("st_sem0"), nc.alloc_semaphore("st_sem1")]
    gxd = [nc.alloc_semaphore("gxd0"), nc.alloc_semaphore("gxd1")]
    idm = nc.alloc_semaphore("idm")
    mmr = nc.alloc_semaphore("mmr")
    act_s = nc.alloc_semaphore("act_s")
    dve_s = nc.alloc_semaphore("dve_s")
    prep_s = nc.alloc_semaphore("prep_s")
    # exchange e increments rsem[e%2] by 14 (7 peers x 2), lsem[e%2] by 16
    rsem = [nc.alloc_semaphore("rsem0"), nc.alloc_semaphore("rsem1")]
    lsem = [nc.alloc_semaphore("lsem0"), nc.alloc_semaphore("lsem1")]
    fin = nc.alloc_semaphore("fin")

    from contextlib import ExitStack

    with ExitStack() as ctx:
        sb = lambda name, shape, d: ctx.enter_context(nc.sbuf_tensor(name, shape, d))
        idx_sb = sb("idx_sb", [128, TT * (NT // 16)], dt.int16)
        wih_sb = sb("wih_sb", [128, KC * G], dt.bfloat16)
        whh_sb = sb("whh_sb", [128, KH * G], dt.bfloat16)
        ident_sb = sb("ident_sb", [128, 128], dt.bfloat16)
        gbias_sb = sb("gbias_sb", [128, NCHUNK], dt.float32)
        slot_sb = sb("slot_sb", [1, 4], dt.int32)
        xt = [sb(f"xt{i}", [128, KC, NT], dt.bfloat16) for i in range(2)]
        stage = [sb(f"stage{i}", [128, TPT * NCHUNK * B], dt.bfloat16) for i in range(2)]
        hg = [sb(f"hg{i}", [128, CORES * B], dt.bfloat16) for i in range(4)]
        hsend = [sb(f"hsend{i}", [128, B], dt.bfloat16) for i in range(2)]
        gxt = [sb(f"gxt{i}", [128, NCHUNK * B], dt.bfloat16) for i in range(2)]
        sg = sb("sg", [128, NCHUNK * B], dt.float32)
        ig_sb = sb("ig_sb", [128, B], dt.float32)
        fc_sb = sb("fc_sb", [128, B], dt.float32)
        thc_sb = sb("thc_sb", [128, B], dt.float32)
        c_sb = sb("c_sb", [128, B], dt.float32)
        hout_sb = sb("hout_sb", [128, B], dt.float32)
        bar_sb = sb("bar_sb", [128, 4], dt.float32)
        psum = [
            ctx.enter_context(nc.psum_tensor(f"ps{i}", [128, 512], dt.float32))
            for i in range(8)
        ]
        block = ctx.enter_context(nc.Block())

        NIDX = NT // 16

        # =========== SYNC engine ===========
        @block.sync
        def _(sy):
            sy.dma_start(
                out=idx_sb.ap().rearrange("p (t c) -> p t c", t=TT),
                in_=idx_d.ap().rearrange("t p c -> p t c"),
            ).then_inc(wload, 16)
            sy.dma_start(out=wih_sb[:, :], in_=wih_d[:, :]).then_inc(wload, 16)
            sy.dma_start(out=whh_sb[:, :], in_=whh_d[:, :]).then_inc(wload, 16)
            sy.dma_start(out=ident_sb[:, :], in_=ident_d[:, :]).then_inc(wload, 16)
            sy.dma_start(out=gbias_sb[:, :], in_=gbias_d[:, :]).then_inc(wload, 16)
            sy.dma_start(out=slot_sb[:, :], in_=slot_d[:, :]).then_inc(wload, 16)

            # phase-1 stores
            for tau in range(TT):
                sy.wait_ge(cp_sem, 4 * tau + 4)
                sy.dma_start(
                    out=gx_d[:, TPT * tau : TPT * (tau + 1), :],
                    in_=stage[tau % 2].ap().rearrange("p (t e) -> p t e", t=TPT),
                ).then_inc(st_sem[tau % 2], 16)

            # phase-2 gx prefetch
            sy.dma_start(out=gxt[0][:, :], in_=gx_d[:, 0, :]).then_inc(gxd[0], 16)
            if nsteps > 1:
                sy.dma_start(out=gxt[1][:, :], in_=gx_d[:, 1, :]).then_inc(gxd[1], 16)
            for t in range(2, nsteps):
                sy.wait_ge(idm, t - 1)
                sy.dma_start(out=gxt[t % 2][:, :], in_=gx_d[:, t, :]).then_inc(gxd[t % 2], 16)

            # final outputs
            sy.wait_ge(dve_s, 1 + 4 * nsteps)
            sy.dma_start(out=out_d[0, :, :], in_=hout_sb[:, :]).then_inc(fin, 16)
            sy.dma_start(out=out_d[1, :, :], in_=c_sb[:, :]).then_inc(fin, 16)
            sy.wait_ge(fin, 32)

        # =========== GPSIMD ===========
        @block.gpsimd
        def _(gp):
            # zero the hg landing buffers BEFORE the barrier: own slot is
            # never written remotely and is consumed with zero weights.
            for b in range(4):
                gp.memset(hg[b][:, :], 0.0)
            gp.memset(bar_sb[:, :], 0.0).then_inc(bar_sem, 1)
            gp.wait_ge(bar_sem, 1)
            gp.wait_ge(wload, 96)  # all constants (incl. slot) loaded
            off = gp.alloc_register("off")
            gp.reg_load(off, slot_sb[0:1, 0:1])
            # barrier: protects remote-sem increments from racing a peer's
            # kernel-start semaphore init (and hg memset).
            gp.dma_start(out=bar_in[:, :], in_=bar_sb[:, :]).then_inc(bardma_sem, 16)
            gp.wait_ge(bardma_sem, 16)
            gp.collective_compute(
                "AllReduce",
                mybir.AluOpType.add,
                ins=[bar_in.ap().opt()],
                outs=[bar_out.ap().opt()],
                replica_groups=[list(range(CORES))],
            ).then_inc(cc_sem, 1)

            # phase-1 embedding gathers
            for tau in range(TT):
                if tau >= 2:
                    gp.wait_ge(mm1, 4 * (tau - 2) + 4)
                gp.dma_gather(
                    out_ap=xt[tau % 2][:, :, :],
                    in_ap=emb_d[:, :],
                    idxs_ap=idx_sb[:, NIDX * tau : NIDX * (tau + 1)],
                    num_idxs=NT,
                    num_idxs_reg=NT,
                    elem_size=EMB,
                    transpose=True,
                ).then_inc(g_sem[tau % 2], 16)

            # phase-2 h exchange: ONE broadcast per step
            gp.wait_ge(cc_sem, 1)
            rd = [None] + [(0, d) for d in range(1, CORES)]
            for t in range(nsteps - 1):
                sl = hg[t % 4][:, 0:B]
                out_ap = AP(sl.tensor, off, sl.ap, sl.const_val,
                            sl.runtime_checks, sl.dep_tracking_offset)
                gp.remote_dma_broadcast(
                    out_ap=out_ap,
                    in_ap=hsend[t % 2][:, :],
                    remote_sem=rsem[t % 2],
                    local_sem=lsem[t % 2],
                    rdests=rd,
                ).then_inc(prep_s, 1)
                gp.wait_ge(prep_s, t + 1)
                if wait_h:
                    gp.wait_ge(dve_s, 1 + 4 * t + 4)  # h(t) written to hsend
                gp.trigger_dma(count=1)
            # drain: all sends fully read before program end (no cross-
            # invocation lsem leakage into freshly initialized semaphores)
            nex = nsteps - 1
            for p in range(2):
                tot = (nex + 1) // 2 if p == 0 else nex // 2
                if tot:
                    gp.wait_ge(lsem[p], 16 * tot)

        # =========== TENSOR engine ===========
        @block.tensor
        def _(te):
            te.wait_ge(wload, 96)
            # ---- phase 1 ----
            for tau in range(TT):
                te.wait_ge(g_sem[tau % 2], 16 * (tau // 2 + 1))
                for cb in range(NCHUNK):
                    pb = psum[(tau % 2) * 4 + cb]
                    if tau >= 2:
                        te.wait_ge(cp_sem, 4 * (tau - 2) + cb + 1)
                    for k in range(KC):
                        mm = te.matmul(
                            pb[:, :],
                            lhsT=wih_sb[:, G * k + 128 * cb : G * k + 128 * (cb + 1)],
                            rhs=xt[tau % 2][:, k, :],
                            start=(k == 0),
                            stop=(k == KC - 1),
                        )
                    mm.then_inc(mm1, 1)

            # ---- phase 2 ----
            for t in range(nsteps):
                P = t % 2
                if wait_gx:
                    te.wait_ge(gxd[t % 2], 16 * (t // 2 + 1))
                if t < 2:
                    te.wait_ge(cp_sem, 4 * TT)
                else:
                    te.wait_ge(act_s, 5 * (t - 2) + 4)  # psum parity reuse
                for cb in range(NCHUNK):
                    mm = te.matmul(
                        psum[P * 4 + cb][:, 0:B],
                        lhsT=ident_sb[:, :],
                        rhs=gxt[P][:, B * cb : B * (cb + 1)],
                        start=True,
                        stop=(t == 0),
                    )
                    if cb == NCHUNK - 1:
                        mm.then_inc(idm, 1)
                if t >= 1:
                    te.wait_ge(dve_s, 1 + 4 * t)  # h(t-1) in hsend[(t-1)%2]
                    if wait_rsem:
                        # all of exchange t-1 arrived (7 peers x 2)
                        te.wait_ge(rsem[(t - 1) % 2], 14 * ((t + 1) // 2))
                    hgb = hg[(t - 1) % 4]
                    for cb in range(NCHUNK):
                        for d in range(KH):
                            src = (
                                hgb[:, B * d : B * (d + 1)]
                                if d < CORES
                                else hsend[(t - 1) % 2][:, :]
                            )
                            mm = te.matmul(
                                psum[P * 4 + cb][:, 0:B],
                                lhsT=whh_sb[:, G * d + 128 * cb : G * d + 128 * (cb + 1)],
                                rhs=src,
                                start=False,
                                stop=(d == KH - 1),
                            )
                        mm.then_inc(mmr, 1)

        # =========== SCALAR engine (ACT) ===========
        @block.scalar
        def _(sc):
            sc.wait_ge(wload, 96)
            # ---- phase 1: psum -> stage (bf16 cast) ----
            for tau in range(TT):
                for cb in range(NCHUNK):
                    sc.wait_ge(mm1, 4 * tau + cb + 1)
                    if tau >= 2:
                        sc.wait_ge(st_sem[tau % 2], 16 * (tau // 2))
                    src = psum[(tau % 2) * 4 + cb].ap().rearrange("p (t b) -> p t b", t=TPT)
                    dst = stage[tau % 2].ap().rearrange(
                        "p (t e b) -> p t e b", t=TPT, e=NCHUNK
                    )[:, :, cb, :]
                    sc.activation(dst, src, AF.Copy).then_inc(cp_sem, 1)

            # ---- phase 2 activations ----
            for t in range(nsteps):
                P = t % 2
                for cb in range(NCHUNK):
                    if t == 0:
                        sc.wait_ge(idm, 1)
                    else:
                        sc.wait_ge(mmr, 4 * (t - 1) + cb + 1)
                    fn = AF.Tanh if cb == 0 else AF.Sigmoid
                    sc.activation(
                        sg[:, B * cb : B * (cb + 1)],
                        psum[P * 4 + cb][:, 0:B],
                        fn,
                        bias=gbias_sb[:, cb : cb + 1],
                    ).then_inc(act_s, 1)
                sc.wait_ge(dve_s, 1 + 4 * t + 3)  # c updated
                sc.activation(thc_sb[:, :], c_sb[:, :], AF.Tanh).then_inc(act_s, 1)

        # =========== VECTOR engine (DVE) ===========
        @block.vector
        def _(ve):
            ve.memset(c_sb[:, :], 0.0).then_inc(dve_s, 1)
            for t in range(nsteps):
                ve.wait_ge(act_s, 5 * t + 2)
                ve.tensor_mul(ig_sb[:, :], sg[:, B : 2 * B], sg[:, 0:B]).then_inc(dve_s, 1)
                ve.wait_ge(act_s, 5 * t + 3)
                ve.wait_ge(dve_s, max(1, 1 + 4 * (t - 1) + 3))
                ve.tensor_mul(fc_sb[:, :], sg[:, 2 * B : 3 * B], c_sb[:, :]).then_inc(dve_s, 1)
                ve.wait_ge(dve_s, 1 + 4 * t + 2)
                ve.tensor_add(c_sb[:, :], ig_sb[:, :], fc_sb[:, :]).then_inc(dve_s, 1)
                ve.wait_ge(act_s, 5 * t + 5)
                if t == nsteps - 1:
                    ve.tensor_mul(hout_sb[:, :], sg[:, 3 * B : 4 * B], thc_sb[:, :]).then_inc(dve_s, 1)
                else:
                    # hsend[t%2] reuse: sends of exchange t-2 done reading it
                    if t >= 2:
                        ve.wait_ge(lsem[t % 2], 16 * (t // 2))
                    ve.tensor_mul(
                        hsend[t % 2][:, :], sg[:, 3 * B : 4 * B], thc_sb[:, :]
                    ).then_inc(dve_s, 1)

    nc.compile()
    return nc


# ---------------------------------------------------------------------------
# host-side input prep
# ---------------------------------------------------------------------------

def prepare_in_maps(source, emb, W_ih, W_hh, b_ih, b_hh, nsteps=S):
    source = np.asarray(source)
    emb = np.asarray(emb, np.float32)
    W_ih = np.asarray(W_ih, np.float32)
    W_hh = np.asarray(W_hh, np.float32)
    b = np.asarray(b_ih, np.float32) + np.asarray(b_hh, np.float32)

    TT = B * nsteps // NT
    emb16 = emb.astype(BF16)
    ident = np.eye(128, dtype=BF16)

    idx = np.zeros([TT, 128, NT // 16], np.int16)
    j = np.arange(NT)
    tprime, bb = j // B, j % B
    for tau in range(TT):
        ids = source[bb, TPT * tau + tprime].astype(np.int16)
        wrapped = ids.reshape(NT // 16, 16).T
        idx[tau] = np.tile(wrapped, (8, 1))

    in_maps = []
    H = HID
    for j_core in range(CORES):
        rows = np.concatenate(
            [
                np.arange(CHUNK_TO_BLOCK[cb] * H + 128 * j_core,
                          CHUNK_TO_BLOCK[cb] * H + 128 * (j_core + 1))
                for cb in range(NCHUNK)
            ]
        )
        Wi = W_ih[rows]  # [512, 1024]
        Wh = W_hh[rows]
        bi = b[rows]

        # w_ih[p, G*k + 128*cb + m] = Wi[128*cb + m, 128*k + p]
        wi4 = Wi.reshape(NCHUNK, 128, KC, 128)          # [cb, m, k, p]
        wih = np.transpose(wi4, (3, 2, 0, 1)).reshape(128, KC * G).astype(BF16)

        # w_hh: 9 chunks; chunk d<8 = hidden block d (slot d of hg),
        # except own block zeroed; chunk 8 = own hidden block (hsend).
        wh4 = Wh.reshape(NCHUNK, 128, KC, 128)          # [cb, m, k, p]
        wh9 = np.zeros((NCHUNK, 128, KH, 128), np.float32)
        for d_slot in range(CORES):
            if d_slot != j_core:
                wh9[:, :, d_slot, :] = wh4[:, :, d_slot, :]
        wh9[:, :, 8, :] = wh4[:, :, j_core, :]
        whh = np.transpose(wh9, (3, 2, 0, 1)).reshape(128, KH * G).astype(BF16)

        gbias = bi.reshape(NCHUNK, 128).T.copy().astype(np.float32)

        in_maps.append(
            {
                "emb16": emb16,
                "idx": idx,
                "w_ih": wih,
                "w_hh": whh,
                "ident": ident,
                "gbias": gbias,
                "slot": np.array([[B * j_core, 0, 0, 0]], np.int32),
            }
        )
    return in_maps


_BUILD_CACHE = {}


def _get_nc(nsteps=S, **kw):
    key = (nsteps, tuple(sorted(kw.items())))
    if key not in _BUILD_CACHE:
        _BUILD_CACHE[key] = build(nsteps, **kw)
    return _BUILD_CACHE[key]


def kernel(source, emb, W_ih, W_hh, b_ih, b_hh, _trace=False):
    from concourse.bass_utils import run_bass_kernel_spmd

    nc = _get_nc()
    in_maps = prepare_in_maps(source, emb, W_ih, W_hh, b_ih, b_hh)
    res = run_bass_kernel_spmd(nc, in_maps, core_ids=list(range(CORES)), trace=_trace)
    outs = [res.results[i]["out"] for i in range(CORES)]
    h = np.concatenate([o[0].T for o in outs], axis=1)
    c = np.concatenate([o[1].T for o in outs], axis=1)
    out = np.stack([h, c]).astype(np.float32)
    if _trace:
        return out, res
    return out


# ---------------------------------------------------------------------------
# dev: multi-core simulation on a reduced problem
# ---------------------------------------------------------------------------

_M = [0, 1, 2, 3, 6, 7, 4, 5]


def _simulate(nsteps=8, check_with_hw=False):
    from concourse import bass_interp, libnrt

    libnrt.get_trn2_nc_mapping.cache_clear()
    fake_map = {(d, i): _M[i] for d in range(16) for i in range(8)}
    libnrt.get_trn2_nc_mapping = lambda: fake_map
    libnrt.nc_to_real_nc = lambda dev, i: fake_map[(dev, i)]
    bass_interp.nc_to_real_nc = libnrt.nc_to_real_nc
    bass_interp.pnc_id_to_device_and_real_nc_index = (
        lambda core_id: (core_id // 8, fake_map[(core_id // 8, core_id % 8)])
    )
    fake_rid = {d: d for d in range(16)}
    libnrt.get_device_id_to_routing_id_mapping = lambda: fake_rid
    bass_interp.get_device_id_to_routing_id_mapping = lambda: fake_rid

    rng = np.random.default_rng(0)
    source = rng.integers(0, VOCAB, (B, nsteps)).astype(np.int32)
    emb = rng.standard_normal((VOCAB, EMB), np.float32)
    W_ih = (rng.standard_normal((4 * HID, EMB), np.float32) / np.sqrt(EMB)).astype(np.float32)
    W_hh = (rng.standard_normal((4 * HID, HID), np.float32) / np.sqrt(HID)).astype(np.float32)
    b_ih = np.zeros(4 * HID, np.float32)
    b_hh = np.zeros(4 * HID, np.float32)

    nc = build(nsteps)
    in_maps = prepare_in_maps(source, emb, W_ih, W_hh, b_ih, b_hh, nsteps)

    sim = bass_interp.MultiCoreSim(nc, CORES)
    for i in range(CORES):
        for k, v in in_maps[i].items():
            sim.cores[i].tensor(k)[:] = v
    sim.simulate(check_with_hw=check_with_hw)

    outs = [
        np.array(sim.cores[i].mem_tensor("out")).reshape(2, 128, B)
        for i in range(CORES)
    ]
    h = np.concatenate([o[0].T for o in outs], axis=1)
    c = np.concatenate([o[1].T for o in outs], axis=1)
    actual = np.stack([h, c])

    X = emb[source]
    hh = np.zeros((B, HID), np.float32)
    cc = np.zeros((B, HID), np.float32)
    for t in range(nsteps):
        gates = X[:, t, :] @ W_ih.T + hh @ W_hh.T + b_ih + b_hh
        i_, f_, g_, o_ = np.split(gates, 4, axis=-1)
        i_ = 1 / (1 + np.exp(-i_))
        f_ = 1 / (1 + np.exp(-f_))
        g_ = np.tanh(g_)
        o_ = 1 / (1 + np.exp(-o_))
        cc = f_ * cc + i_ * g_
        hh = o_ * np.tanh(cc)
    expected = np.stack([hh, cc])
    err = np.abs(actual - expected).max() / np.abs(expected).max()
    times = [sim.cores[i].time for i in range(CORES)]
    print(f"sim nsteps={nsteps} absmax_rel_err={err:.3e} sim_time_ns={max(times)}")
    return err


if __name__ == "__main__":
    ns = int(sys.argv[1]) if len(sys.argv) > 1 else 8
    _simulate(ns)


# revision 3
# speedup vs baseline: 3.4557x; 1.4207x over previous
"""LSTM encoder (B=64, S=512, E=H=1024) on 8 trn2 NeuronCores — v2.

v2 exchange: ONE remote_dma_broadcast per step (vs 7 in v1). The
broadcast targets all 7 peers (relative dests (0,1)..(0,7)); the landing
offset inside each receiver's gather buffer comes from a POOL register
loaded per-core (64*logical_id), so every sender lands in its own slot —
sender-indexed addressing without per-core programs. A core's own slice
never crosses the fabric: it is consumed from the send buffer via a 9th
W_hh contraction chunk, and the (never-written, zero-initialized) own
slot is neutralized with zero weights.

Layout per core (gate-dim tensor parallel, as v1):
  - 512 gate rows (4 gates x 128 hidden channels), full batch 64.
  - Phase 1: dma_gather embedding -> X^T tiles; gx = W_ih_local @ X^T
    streamed to DRAM as bf16.
  - Phase 2: per step: identity-matmul loads gx_t into PSUM; 9x4 W_hh
    matmuls accumulate (8 hg slots + hsend); sigmoid/tanh on ACT; cell
    update on DVE; h slice -> hsend[t%2]; one broadcast -> peers'
    hg[t%4] at slot 64*myid.
"""

import os
import sys

sys.path.insert(0, "/opt/trn_rl_repo")

import numpy as np
import ml_dtypes

import concourse.bass as bass
import concourse.bacc as bacc
import concourse.mybir as mybir
from concourse.ap import AP

BF16 = ml_dtypes.bfloat16
AF = mybir.ActivationFunctionType
dt = mybir.dt

VOCAB, EMB, HID = 32000, 1024, 1024
B = 64
S = 512
CORES = 8
KC = 8            # contraction chunks of 128 (phase 1)
KH = 9            # phase-2 W_hh chunks: 8 hg slots + own (hsend)
NCHUNK = 4        # gate chunks per core (order: g, i, f, o)
G = NCHUNK * 128  # 512 gate rows per core
NT = 512          # tokens per phase-1 tile
TPT = NT // B     # timesteps per phase-1 tile (8)
# pytorch gate blocks in W rows: i, f, g, o ; our chunk order: g, i, f, o
CHUNK_TO_BLOCK = [2, 0, 1, 3]


def build(nc_steps=S, wait_rsem=True, wait_h=True, wait_gx=True, warm=0):
    nsteps = nc_steps
    TT = B * nsteps // NT
    assert B * nsteps % NT == 0

    nc = bacc.Bacc(None, target_bir_lowering=False)

    # ---- kernel I/O (per core) ----
    emb_d = nc.declare_dram_parameter("emb16", [VOCAB, EMB], dt.bfloat16, isOutput=False)
    idx_d = nc.declare_dram_parameter("idx", [TT, 128, NT // 16], dt.int16, isOutput=False)
    wih_d = nc.declare_dram_parameter("w_ih", [128, KC * G], dt.bfloat16, isOutput=False)
    whh_d = nc.declare_dram_parameter("w_hh", [128, KH * G], dt.bfloat16, isOutput=False)
    ident_d = nc.declare_dram_parameter("ident", [128, 128], dt.bfloat16, isOutput=False)
    gbias_d = nc.declare_dram_parameter("gbias", [128, NCHUNK], dt.float32, isOutput=False)
    slot_d = nc.declare_dram_parameter("slot", [1, 4], dt.int32, isOutput=False)
    out_d = nc.declare_dram_parameter("out", [2, 128, B], dt.float32, isOutput=True)

    # ---- DRAM scratch ----
    gx_d = nc.dram_tensor("gx", [128, nsteps, NCHUNK * B], dt.bfloat16)
    bar_in = nc.dram_tensor("bar_in", [128, 4], dt.float32)
    bar_out = nc.dram_tensor("bar_out", [128, 4], dt.float32, addr_space="Shared")

    # ---- semaphores ----
    cc_sem = nc.alloc_semaphore("cc_sem")
    bar_sem = nc.alloc_semaphore("bar_sem")
    bardma_sem = nc.alloc_semaphore("bardma_sem")
    wload = nc.alloc_semaphore("wload")
    g_sem = [nc.alloc_semaphore("g_sem0"), nc.alloc_semaphore("g_sem1")]
    mm1 = nc.alloc_semaphore("mm1")
    cp_sem = nc.alloc_semaphore("cp_sem")
    st_sem = [nc.alloc_semaphore("st_sem0"), nc.alloc_semaphore("st_sem1")]
    gxd = [nc.alloc_semaphore("gxd0"), nc.alloc_semaphore("gxd1")]
    idm = nc.alloc_semaphore("idm")
    mmr = nc.alloc_semaphore("mmr")
    act_s = nc.alloc_semaphore("act_s")
    dve_s = nc.alloc_semaphore("dve_s")
    prep_s = nc.alloc_semaphore("prep_s")
    # exchange e increments rsem[e%2] by 14 (7 peers x 2), lsem[e%2] by 16
    rsem = [nc.alloc_semaphore("rsem0"), nc.alloc_semaphore("rsem1")]
    lsem = [nc.alloc_semaphore("lsem0"), nc.alloc_semaphore("lsem1")]
    fin = nc.alloc_semaphore("fin")

    from contextlib import ExitStack

    with ExitStack() as ctx:
        sb = lambda name, shape, d: ctx.enter_context(nc.sbuf_tensor(name, shape, d))
        idx_sb = sb("idx_sb", [128, TT * (NT // 16)], dt.int16)
        wih_sb = sb("wih_sb", [128, KC * G], dt.bfloat16)
        whh_sb = sb("whh_sb", [128, KH * G], dt.bfloat16)
        ident_sb = sb("ident_sb", [128, 128], dt.bfloat16)
        gbias_sb = sb("gbias_sb", [128, NCHUNK], dt.float32)
        slot_sb = sb("slot_sb", [1, 4], dt.int32)
        xt = [sb(f"xt{i}", [128, KC, NT], dt.bfloat16) for i in range(2)]
        stage = [sb(f"stage{i}", [128, TPT * NCHUNK * B], dt.bfloat16) for i in range(2)]
        hg = [sb(f"hg{i}", [128, CORES * B], dt.bfloat16) for i in range(4)]
        hsend = [sb(f"hsend{i}", [128, B], dt.bfloat16) for i in range(2)]
        gxt = [sb(f"gxt{i}", [128, NCHUNK * B], dt.bfloat16) for i in range(2)]
        sg = sb("sg", [128, NCHUNK * B], dt.float32)
        ig_sb = sb("ig_sb", [128, B], dt.float32)
        fc_sb = sb("fc_sb", [128, B], dt.float32)
        thc_sb = sb("thc_sb", [128, B], dt.float32)
        c_sb = sb("c_sb", [128, B], dt.float32)
        hout_sb = sb("hout_sb", [128, B], dt.float32)
        bar_sb = sb("bar_sb", [128, 4], dt.float32)
        psum = [
            ctx.enter_context(nc.psum_tensor(f"ps{i}", [128, 512], dt.float32))
            for i in range(8)
        ]
        block = ctx.enter_context(nc.Block())

        NIDX = NT // 16

        # =========== SYNC engine ===========
        @block.sync
        def _(sy):
            sy.dma_start(
                out=idx_sb.ap().rearrange("p (t c) -> p t c", t=TT),
                in_=idx_d.ap().rearrange("t p c -> p t c"),
            ).then_inc(wload, 16)
            sy.dma_start(out=wih_sb[:, :], in_=wih_d[:, :]).then_inc(wload, 16)
            sy.dma_start(out=whh_sb[:, :], in_=whh_d[:, :]).then_inc(wload, 16)
            sy.dma_start(out=ident_sb[:, :], in_=ident_d[:, :]).then_inc(wload, 16)
            sy.dma_start(out=gbias_sb[:, :], in_=gbias_d[:, :]).then_inc(wload, 16)
            sy.dma_start(out=slot_sb[:, :], in_=slot_d[:, :]).then_inc(wload, 16)

            # phase-1 stores
            for tau in range(TT):
                sy.wait_ge(cp_sem, 4 * tau + 4)
                sy.dma_start(
                    out=gx_d[:, TPT * tau : TPT * (tau + 1), :],
                    in_=stage[tau % 2].ap().rearrange("p (t e) -> p t e", t=TPT),
                ).then_inc(st_sem[tau % 2], 16)

            # phase-2 gx prefetch
            sy.dma_start(out=gxt[0][:, :], in_=gx_d[:, 0, :]).then_inc(gxd[0], 16)
            if nsteps > 1:
                sy.dma_start(out=gxt[1][:, :], in_=gx_d[:, 1, :]).then_inc(gxd[1], 16)
            for t in range(2, nsteps):
                sy.wait_ge(idm, t - 1)
                sy.dma_start(out=gxt[t % 2][:, :], in_=gx_d[:, t, :]).then_inc(gxd[t % 2], 16)

            # final outputs
            sy.wait_ge(dve_s, 1 + 4 * nsteps)
            sy.dma_start(out=out_d[0, :, :], in_=hout_sb[:, :]).then_inc(fin, 16)
            sy.dma_start(out=out_d[1, :, :], in_=c_sb[:, :]).then_inc(fin, 16)
            sy.wait_ge(fin, 32)

        # =========== GPSIMD ===========
        @block.gpsimd
        def _(gp):
            # zero the hg landing buffers BEFORE the barrier: own slot is
            # never written remotely and is consumed with zero weights.
            for b in range(4):
                gp.memset(hg[b][:, :], 0.0)
            gp.memset(bar_sb[:, :], 0.0).then_inc(bar_sem, 1)
            gp.wait_ge(bar_sem, 1)
            gp.wait_ge(wload, 96)  # all constants (incl. slot) loaded
            off = gp.alloc_register("off")
            gp.reg_load(off, slot_sb[0:1, 0:1])
            # barrier: protects remote-sem increments from racing a peer's
            # kernel-start semaphore init (and hg memset).
            gp.dma_start(out=bar_in[:, :], in_=bar_sb[:, :]).then_inc(bardma_sem, 16)
            gp.wait_ge(bardma_sem, 16)
            gp.collective_compute(
                "AllReduce",
                mybir.AluOpType.add,
                ins=[bar_in.ap().opt()],
                outs=[bar_out.ap().opt()],
                replica_groups=[list(range(CORES))],
            ).then_inc(cc_sem, 1)

            # phase-1 embedding gathers
            for tau in range(TT):
                if tau >= 2:
                    gp.wait_ge(mm1, 4 * (tau - 2) + 4)
                gp.dma_gather(
                    out_ap=xt[tau % 2][:, :, :],
                    in_ap=emb_d[:, :],
                    idxs_ap=idx_sb[:, NIDX * tau : NIDX * (tau + 1)],
                    num_idxs=NT,
                    num_idxs_reg=NT,
                    elem_size=EMB,
                    transpose=True,
                ).then_inc(g_sem[tau % 2], 16)

            # phase-2 h exchange: ONE broadcast per step
            gp.wait_ge(cc_sem, 1)
            rd = [None] + [(0, d) for d in range(1, CORES)]
            for t in range(nsteps - 1):
                sl = hg[t % 4][:, 0:B]
                out_ap = AP(sl.tensor, off, sl.ap, sl.const_val,
                            sl.runtime_checks, sl.dep_tracking_offset)
                gp.remote_dma_broadcast(
                    out_ap=out_ap,
                    in_ap=hsend[t % 2][:, :],
                    remote_sem=rsem[t % 2],
                    local_sem=lsem[t % 2],
                    rdests=rd,
                ).then_inc(prep_s, 1)
                gp.wait_ge(prep_s, t + 1)
                if wait_h:
                    gp.wait_ge(dve_s, 1 + 4 * t + 4)  # h(t) written to hsend
                gp.trigger_dma(count=1)
            # drain: all sends fully read before program end (no cross-
            # invocation lsem leakage into freshly initialized semaphores)
            nex = nsteps - 1
            for p in range(2):
                tot = (nex + 1) // 2 if p == 0 else nex // 2
                if tot:
                    gp.wait_ge(lsem[p], 16 * tot)

        # =========== TENSOR engine ===========
        @block.tensor
        def _(te):
            te.wait_ge(wload, 96)
            # ---- phase 1 ----
            for tau in range(TT):
                te.wait_ge(g_sem[tau % 2], 16 * (tau // 2 + 1))
                for cb in range(NCHUNK):
                    pb = psum[(tau % 2) * 4 + cb]
                    if tau >= 2:
                        te.wait_ge(cp_sem, 4 * (tau - 2) + cb + 1)
                    for k in range(KC):
                        mm = te.matmul(
                            pb[:, :],
                            lhsT=wih_sb[:, G * k + 128 * cb : G * k + 128 * (cb + 1)],
                            rhs=xt[tau % 2][:, k, :],
                            start=(k == 0),
                            stop=(k == KC - 1),
                        )
                    mm.then_inc(mm1, 1)

            # ---- phase 2 ----
            for t in range(nsteps):
                P = t % 2
                if wait_gx:
                    te.wait_ge(gxd[t % 2], 16 * (t // 2 + 1))
                if t < 2:
                    te.wait_ge(cp_sem, 4 * TT)
                else:
                    te.wait_ge(act_s, 5 * (t - 2) + 4)  # psum parity reuse
                for cb in range(NCHUNK):
                    mm = te.matmul(
                        psum[P * 4 + cb][:, 0:B],
                        lhsT=ident_sb[:, :],
                        rhs=gxt[P][:, B * cb : B * (cb + 1)],
                        start=True,
                        stop=(t == 0),
                    )
                    if cb == NCHUNK - 1:
                        mm.then_inc(idm, 1)
                if t >= 1:
                    te.wait_ge(dve_s, 1 + 4 * t)  # h(t-1) in hsend[(t-1)%2]
                    # own-slice mms (d=8, reads hsend) need no exchange:
                    # issue them inside the exchange-wait window so only the
                    # 8 hg-slot mms per bank sit on the exposed critical path
                    for cb in range(NCHUNK):
                        te.matmul(
                            psum[P * 4 + cb][:, 0:B],
                            lhsT=whh_sb[:, G * 8 + 128 * cb : G * 8 + 128 * (cb + 1)],
                            rhs=hsend[(t - 1) % 2][:, :],
                            start=False,
                            stop=False,
                        )
                    if wait_rsem:
                        # all of exchange t-1 arrived (7 peers x 2)
                        te.wait_ge(rsem[(t - 1) % 2], 14 * ((t + 1) // 2))
                    hgb = hg[(t - 1) % 4]
                    for cb in range(NCHUNK):
                        for d in range(CORES):
                            mm = te.matmul(
                                psum[P * 4 + cb][:, 0:B],
                                lhsT=whh_sb[:, G * d + 128 * cb : G * d + 128 * (cb + 1)],
                                rhs=hgb[:, B * d : B * (d + 1)],
                                start=False,
                                stop=(d == CORES - 1),
                            )
                        mm.then_inc(mmr, 1)
                # dummy weight loads: keep the PE array busy through the
                # act/dve/exchange window so the HAM clock gate holds
                # 2.4 GHz (idle PE decays to 1.2 GHz). ldweights touches no
                # PSUM (concurrent PE-write + ACT-read of a bank is a fatal
                # PSUM collision), and every real matmul reloads its own
                # stationary operand, so these are side-effect free.
                for w in range(warm):
                    te.ldweights(whh_sb[:, 128 * (w % 32) : 128 * (w % 32 + 1)])

        # =========== SCALAR engine (ACT) ===========
        @block.scalar
        def _(sc):
            sc.wait_ge(wload, 96)
            # ---- phase 1: psum -> stage (bf16 cast) ----
            for tau in range(TT):
                for cb in range(NCHUNK):
                    sc.wait_ge(mm1, 4 * tau + cb + 1)
                    if tau >= 2:
                        sc.wait_ge(st_sem[tau % 2], 16 * (tau // 2))
                    src = psum[(tau % 2) * 4 + cb].ap().rearrange("p (t b) -> p t b", t=TPT)
                    dst = stage[tau % 2].ap().rearrange(
                        "p (t e b) -> p t e b", t=TPT, e=NCHUNK
                    )[:, :, cb, :]
                    sc.activation(dst, src, AF.Copy).then_inc(cp_sem, 1)

            # ---- phase 2 activations ----
            for t in range(nsteps):
                P = t % 2
                for cb in range(NCHUNK):
                    if t == 0:
                        sc.wait_ge(idm, 1)
                    else:
                        sc.wait_ge(mmr, 4 * (t - 1) + cb + 1)
                    fn = AF.Tanh if cb == 0 else AF.Sigmoid
                    sc.activation(
                        sg[:, B * cb : B * (cb + 1)],
                        psum[P * 4 + cb][:, 0:B],
                        fn,
                        bias=gbias_sb[:, cb : cb + 1],
                    ).then_inc(act_s, 1)
                sc.wait_ge(dve_s, 1 + 4 * t + 3)  # c updated
                sc.activation(thc_sb[:, :], c_sb[:, :], AF.Tanh).then_inc(act_s, 1)

        # =========== VECTOR engine (DVE) ===========
        @block.vector
        def _(ve):
            ve.memset(c_sb[:, :], 0.0).then_inc(dve_s, 1)
            for t in range(nsteps):
                ve.wait_ge(act_s, 5 * t + 2)
                ve.tensor_mul(ig_sb[:, :], sg[:, B : 2 * B], sg[:, 0:B]).then_inc(dve_s, 1)
                ve.wait_ge(act_s, 5 * t + 3)
                ve.wait_ge(dve_s, max(1, 1 + 4 * (t - 1) + 3))
                ve.tensor_mul(fc_sb[:, :], sg[:, 2 * B : 3 * B], c_sb[:, :]).then_inc(dve_s, 1)
                ve.wait_ge(dve_s, 1 + 4 * t + 2)
                ve.tensor_add(c_sb[:, :], ig_sb[:, :], fc_sb[:, :]).then_inc(dve_s, 1)
                ve.wait_ge(act_s, 5 * t + 5)
                if t == nsteps - 1:
                    ve.tensor_mul(hout_sb[:, :], sg[:, 3 * B : 4 * B], thc_sb[:, :]).then_inc(dve_s, 1)
                else:
                    # hsend[t%2] reuse: sends of exchange t-2 done reading it
                    if t >= 2:
                        ve.wait_ge(lsem[t % 2], 16 * (t // 2))
                    ve.tensor_mul(
                        hsend[t % 2][:, :], sg[:, 3 * B : 4 * B], thc_sb[:, :]
                    ).then_inc(dve_s, 1)

    nc.compile()
    return nc


# ---------------------------------------------------------------------------
# host-side input prep
# ---------------------------------------------------------------------------

def prepare_in_maps(source, emb, W_ih, W_hh, b_ih, b_hh, nsteps=S):
    source = np.asarray(source)
    emb = np.asarray(emb, np.float32)
    W_ih = np.asarray(W_ih, np.float32)
    W_hh = np.asarray(W_hh, np.float32)
    b = np.asarray(b_ih, np.float32) + np.asarray(b_hh, np.float32)

    TT = B * nsteps // NT
    emb16 = emb.astype(BF16)
    ident = np.eye(128, dtype=BF16)

    idx = np.zeros([TT, 128, NT // 16], np.int16)
    j = np.arange(NT)
    tprime, bb = j // B, j % B
    for tau in range(TT):
        ids = source[bb, TPT * tau + tprime].astype(np.int16)
        wrapped = ids.reshape(NT // 16, 16).T
        idx[tau] = np.tile(wrapped, (8, 1))

    in_maps = []
    H = HID
    for j_core in range(CORES):
        rows = np.concatenate(
            [
                np.arange(CHUNK_TO_BLOCK[cb] * H + 128 * j_core,
                          CHUNK_TO_BLOCK[cb] * H + 128 * (j_core + 1))
                for cb in range(NCHUNK)
            ]
        )
        Wi = W_ih[rows]  # [512, 1024]
        Wh = W_hh[rows]
        bi = b[rows]

        # w_ih[p, G*k + 128*cb + m] = Wi[128*cb + m, 128*k + p]
        wi4 = Wi.reshape(NCHUNK, 128, KC, 128)          # [cb, m, k, p]
        wih = np.transpose(wi4, (3, 2, 0, 1)).reshape(128, KC * G).astype(BF16)

        # w_hh: 9 chunks; chunk d<8 = hidden block d (slot d of hg),
        # except own block zeroed; chunk 8 = own hidden block (hsend).
        wh4 = Wh.reshape(NCHUNK, 128, KC, 128)          # [cb, m, k, p]
        wh9 = np.zeros((NCHUNK, 128, KH, 128), np.float32)
        for d_slot in range(CORES):
            if d_slot != j_core:
                wh9[:, :, d_slot, :] = wh4[:, :, d_slot, :]
        wh9[:, :, 8, :] = wh4[:, :, j_core, :]
        whh = np.transpose(wh9, (3, 2, 0, 1)).reshape(128, KH * G).astype(BF16)

        gbias = bi.reshape(NCHUNK, 128).T.copy().astype(np.float32)

        in_maps.append(
            {
                "emb16": emb16,
                "idx": idx,
                "w_ih": wih,
                "w_hh": whh,
                "ident": ident,
                "gbias": gbias,
                "slot": np.array([[B * j_core, 0, 0, 0]], np.int32),
            }
        )
    return in_maps


_BUILD_CACHE = {}


def _get_nc(nsteps=S, **kw):
    key = (nsteps, tuple(sorted(kw.items())))
    if key not in _BUILD_CACHE:
        _BUILD_CACHE[key] = build(nsteps, **kw)
    return _BUILD_CACHE[key]


def kernel(source, emb, W_ih, W_hh, b_ih, b_hh, _trace=False):
    from concourse.bass_utils import run_bass_kernel_spmd

    nc = _get_nc()
    in_maps = prepare_in_maps(source, emb, W_ih, W_hh, b_ih, b_hh)
    res = run_bass_kernel_spmd(nc, in_maps, core_ids=list(range(CORES)), trace=_trace)
    outs = [res.results[i]["out"] for i in range(CORES)]
    h = np.concatenate([o[0].T for o in outs], axis=1)
    c = np.concatenate([o[1].T for o in outs], axis=1)
    out = np.stack([h, c]).astype(np.float32)
    if _trace:
        return out, res
    return out


# ---------------------------------------------------------------------------
# dev: multi-core simulation on a reduced problem
# ---------------------------------------------------------------------------

_M = [0, 1, 2, 3, 6, 7, 4, 5]


def _simulate(nsteps=8, check_with_hw=False):
    from concourse import bass_interp, libnrt

    libnrt.get_trn2_nc_mapping.cache_clear()
    fake_map = {(d, i): _M[i] for d in range(16) for i in range(8)}
    libnrt.get_trn2_nc_mapping = lambda: fake_map
    libnrt.nc_to_real_nc = lambda dev, i: fake_map[(dev, i)]
    bass_interp.nc_to_real_nc = libnrt.nc_to_real_nc
    bass_interp.pnc_id_to_device_and_real_nc_index = (
        lambda core_id: (core_id // 8, fake_map[(core_id // 8, core_id % 8)])
    )
    fake_rid = {d: d for d in range(16)}
    libnrt.get_device_id_to_routing_id_mapping = lambda: fake_rid
    bass_interp.get_device_id_to_routing_id_mapping = lambda: fake_rid

    rng = np.random.default_rng(0)
    source = rng.integers(0, VOCAB, (B, nsteps)).astype(np.int32)
    emb = rng.standard_normal((VOCAB, EMB), np.float32)
    W_ih = (rng.standard_normal((4 * HID, EMB), np.float32) / np.sqrt(EMB)).astype(np.float32)
    W_hh = (rng.standard_normal((4 * HID, HID), np.float32) / np.sqrt(HID)).astype(np.float32)
    b_ih = np.zeros(4 * HID, np.float32)
    b_hh = np.zeros(4 * HID, np.float32)

    nc = build(nsteps)
    in_maps = prepare_in_maps(source, emb, W_ih, W_hh, b_ih, b_hh, nsteps)

    sim = bass_interp.MultiCoreSim(nc, CORES)
    for i in range(CORES):
        for k, v in in_maps[i].items():
            sim.cores[i].tensor(k)[:] = v
    sim.simulate(check_with_hw=check_with_hw)

    outs = [
        np.array(sim.cores[i].mem_tensor("out")).reshape(2, 128, B)
        for i in range(CORES)
    ]
    h = np.concatenate([o[0].T for o in outs], axis=1)
    c = np.concatenate([o[1].T for o in outs], axis=1)
    actual = np.stack([h, c])

    X = emb[source]
    hh = np.zeros((B, HID), np.float32)
    cc = np.zeros((B, HID), np.float32)
    for t in range(nsteps):
        gates = X[:, t, :] @ W_ih.T + hh @ W_hh.T + b_ih + b_hh
        i_, f_, g_, o_ = np.split(gates, 4, axis=-1)
        i_ = 1 / (1 + np.exp(-i_))
        f_ = 1 / (1 + np.exp(-f_))
        g_ = np.tanh(g_)
        o_ = 1 / (1 + np.exp(-o_))
        cc = f_ * cc + i_ * g_
        hh = o_ * np.tanh(cc)
    expected = np.stack([hh, cc])
    err = np.abs(actual - expected).max() / np.abs(expected).max()
    times = [sim.cores[i].time for i in range(CORES)]
    print(f"sim nsteps={nsteps} absmax_rel_err={err:.3e} sim_time_ns={max(times)}")
    return err


if __name__ == "__main__":
    ns = int(sys.argv[1]) if len(sys.argv) > 1 else 8
    _simulate(ns)
